# revision 20
# baseline (speedup 1.0000x reference)
"""CLUB loss kernel for Trainium2, 8-core data-parallel SPMD (i-major, fp16).

Math: with flat_x (N,D) [from x (B,D,H,W) -> (B*H*W, D)], v = exp(-p_logvar),
  loss = (-0.5/N) * [ A - 2B - dot(m2, V) + 2*dot(m1, W) ]
where
  A = sum_{i,d} x^2 v      B = sum_{i,d} x mu v
  V_d = sum_i v            W_d = sum_i mu v
  S1_d = sum_i x           S2_d = sum_i x^2     m1 = S1/N, m2 = S2/N
Per-core partials; tiny cross-core reduction + final dots on host in f64.

Layout: i-major (partition = token i, free = d).  mu/lv stream in their
natural (N, D) layout -- no transposes.  Only x (natural d-major) is
PE-transposed (64 fp16 identity matmuls/core, ~105ns each).  ALL six
reductions (V/W/S2/S1/Avec/Bvec, each a per-d column sum over i) are
ones-vector matmuls on the PE, PSUM-accumulated across the 16 i-tiles;
A/B are host sums of Avec/Bvec.

fp16 (not bf16): 8x finer mantissa at identical DVE/PE speed; all
intermediates (v<150, w<1k, a,b<6k) fit fp16 range.  HW-measured op
costs that drive this structure: DVE tensor_tensor f16 = 426ns (2x mode;
scalar_tensor_tensor is stuck at 1x/690ns, tensor_tensor_reduce crashes
the runtime), ACT activation = (N+352)/1.2 any dtype, fp16 transposes
~105ns effective, ones-MM ~220-430ns.

Engines/tile (128 i x 512 d): PE 4 transposes + 6 ones-chain MMs;
ACT exp (v, f16) + square (xx from PSUM xT, f16); DVE copy xTb (f16,
from PSUM) + w=mu*v + a=xx*v + b=w*xTb (all tensor_tensor f16 2x).

DMA: x and mu need fp32->fp16 casts so they ride the SWDGE (gpsimd)
ring, ordered by need [xh(group), mu, mu, ...]; lv (fp32, no cast) on
the two HWDGE rings (sync/scalar), throttled to consumption rate by
lv pool recycling (bufs=3) so the SWDGE ring gets the early bandwidth.
"""

import sys

import numpy as np

for _p in ("/opt/trn_rl_repo",):
    if _p not in sys.path:
        sys.path.append(_p)

B, D, H, W = 16, 512, 32, 32
HW = H * W
N = B * HW
NCORES = 8
BLKB = B // NCORES          # b-blocks per core (2)
ROWS = N // NCORES          # rows per core (2048)
NT = ROWS // 128            # 128-row i-tiles per core (16)
NDC = D // 128              # d chunks (4)
NG = 4                      # x groups per core (b-block halves), 4 tiles each

_prog_cache = {}


def build_program():
    import concourse.bacc as bacc
    import concourse.tile as tile
    from concourse import mybir

    f32 = mybir.dt.float32
    f16 = mybir.dt.float16
    AF = mybir.ActivationFunctionType
    OP = mybir.AluOpType

    nc = bacc.Bacc(
        "TRN2",
        target_bir_lowering=False,
        debug=False,
        enable_asserts=False,
        num_devices=NCORES,
    )

    x_d = nc.dram_tensor("x_s", (BLKB, D, HW), f32, kind="ExternalInput").ap()
    mu_d = nc.dram_tensor("mu_s", (ROWS, D), f32, kind="ExternalInput").ap()
    lv_d = nc.dram_tensor("lv_s", (ROWS, D), f32, kind="ExternalInput").ap()
    id_d = nc.dram_tensor("identh", (128, 128), f16, kind="ExternalInput").ap()

    # o_vec rows (after host reshape to (5, D)): [V, W, S2, Avec, Bvec]
    o_vec = nc.dram_tensor("o_vec", (1, 5 * D), f32, kind="ExternalOutput").ap()
    # S1 partials: col 4t+dc (boot tiles t<4), col 16+4*(g-1)+dc (groups)
    o_s1 = nc.dram_tensor("o_s1", (128, 28), f32, kind="ExternalOutput").ap()

    with tile.TileContext(nc) as tc:
        with (
            tc.tile_pool(name="const", bufs=1) as constp,
            tc.tile_pool(name="xh", bufs=3) as xp,
            tc.tile_pool(name="lvsl", bufs=3) as lvp,
            tc.tile_pool(name="musl", bufs=3) as mup,
            tc.tile_pool(name="elem", bufs=3) as ep,
            tc.tile_pool(name="accum", bufs=1) as accp,
            tc.tile_pool(name="psum", bufs=3, space="PSUM") as pp,
            tc.tile_pool(name="psv", bufs=1, space="PSUM") as pvp,
        ):
            identh = constp.tile([128, 128], f16, tag="idh", name="idh")
            ones = constp.tile([128, 1], f16, tag="ones", name="ones")
            nc.vector.memset(ones[:], 1.0)

            vws = [
                pvp.tile([1, D], f32, tag=f"vws{q}", name=f"vws{q}")
                for q in range(5)
            ]
            evac = accp.tile([1, 5 * D], f32, tag="evac", name="evac")
            accS1 = accp.tile([128, 28], f32, tag="accs1", name="accs1")

            lv_slabs = {}   # slab s: i-tiles 2s, 2s+1 (fp32, natural)
            mu_slabs = {}   # slab s: i-tiles 2s, 2s+1 (f16 cast, natural)
            xhalf = {}      # group g: (128, NDC*512) f16, d-major half-block

            def load_lv(s, eng):
                t_ = lvp.tile([128, 2 * D], f32, tag="lv", name="lv_sl")
                rows = lv_d[256 * s : 256 * (s + 1), :]
                eng.dma_start(t_[:], rows.rearrange("(g p) f -> p g f", p=128))
                lv_slabs[s] = t_

            def load_mu(s):
                t_ = mup.tile([128, 2 * D], f16, tag="mu", name="mu_sl")
                rows = mu_d[256 * s : 256 * (s + 1), :]
                nc.gpsimd.dma_start(t_[:], rows.rearrange("(g p) f -> p g f", p=128))
                mu_slabs[s] = t_

            xtile = {}      # t (<4): (128, NDC*128) f16 single-tile x

            def load_xh(g):
                b, h = divmod(g, 2)
                t_ = xp.tile([128, NDC * 512], f16, tag="xh", name="xh")
                src = x_d[b, :, 512 * h : 512 * (h + 1)]
                nc.gpsimd.dma_start(t_[:], src.rearrange("(g p) f -> p g f", p=128))
                xhalf[g] = t_

            def load_xt(t):
                # per-tile x load (256 KB read) for the boot group: lets
                # tile t's transposes start without waiting for a full
                # 1 MB group
                t_ = xp.tile([128, NDC * 128], f16, tag="xt", name="xt")
                src = x_d[0, :, 128 * t : 128 * (t + 1)]
                nc.gpsimd.dma_start(t_[:], src.rearrange("(g p) f -> p g f", p=128))
                xtile[t] = t_

            # ---- DMA issue ----
            # SWDGE ring (casts): by-need order; HW rings: lv, throttled
            # by lvp recycling so they don't front-run the SWDGE stream.
            nc.sync.dma_start(identh[:], id_d[:])
            load_xt(0)
            load_mu(0)
            load_xt(1)
            load_xt(2)
            load_xt(3)
            load_mu(1)
            load_xh(1)
            load_mu(2)
            load_mu(3)
            load_xh(2)
            load_mu(4)
            load_mu(5)
            load_xh(3)
            load_mu(6)
            load_mu(7)
            for s in range(8):
                load_lv(s, nc.sync if s % 2 == 0 else nc.scalar)

            # ---- compute ----
            def transposes(t):
                g, jj = divmod(t, 4)
                xT = pp.tile([128, D], f16, tag="xT", name="xT")
                for dc in range(NDC):
                    if t < 4:
                        src = xtile[t][:, 128 * dc : 128 * (dc + 1)]
                    else:
                        col = 512 * dc + 128 * jj
                        src = xhalf[g][:, col : col + 128]
                    nc.tensor.matmul(
                        xT[:, 128 * dc : 128 * (dc + 1)],
                        src,
                        identh[:],
                        is_transpose=True,
                        start=(dc == 0),
                        stop=(dc == NDC - 1),
                    )
                return xT

            xT_t = transposes(0)
            for t in range(NT):
                first, last = (t == 0), (t == NT - 1)
                xT = xT_t

                # ACT: v = exp(-lv) f16; xx = (xT)^2 f16 (evacuates PSUM)
                lvsl = lv_slabs[t // 2][:, D * (t % 2) : D * (t % 2 + 1)]
                v_t = ep.tile([128, D], f16, tag="v", name="v_t")
                nc.scalar.activation(v_t[:], lvsl, AF.Exp, scale=-1.0)
                xx = ep.tile([128, D], f16, tag="xx", name="xx")
                nc.scalar.activation(xx[:], xT[:], AF.Square)

                # DVE: products (tensor_tensor, 2x); b reads xT from PSUM
                musl = mu_slabs[t // 2][:, D * (t % 2) : D * (t % 2 + 1)]
                w_t = ep.tile([128, D], f16, tag="w", name="w_t")
                nc.vector.tensor_tensor(w_t[:], musl, v_t[:], OP.mult)
                a_t = ep.tile([128, D], f16, tag="a", name="a_t")
                nc.vector.tensor_tensor(a_t[:], xx[:], v_t[:], OP.mult)
                b_t = ep.tile([128, D], f16, tag="b", name="b_t")
                nc.vector.tensor_tensor(b_t[:], w_t[:], xT[:], OP.mult)

                # DVE: S1 partials -- free-axis accums over natural-layout x
                if t < 4:
                    for dc in range(NDC):
                        s1scr = ep.tile([128, 128], f16, tag="s1s", name="s1s")
                        nc.vector.tensor_scalar(
                            s1scr[:], xtile[t][:, 128 * dc : 128 * (dc + 1)],
                            1.0, 0.0, OP.mult, OP.add,
                            accum_out=accS1[:, 4 * t + dc : 4 * t + dc + 1],
                        )
                else:
                    g, jj = divmod(t, 4)
                    dc = jj  # one chunk of group g per tile
                    s1scr = ep.tile([128, D], f16, tag="s1g", name="s1g")
                    nc.vector.tensor_scalar(
                        s1scr[:], xhalf[g][:, 512 * dc : 512 * (dc + 1)],
                        1.0, 0.0, OP.mult, OP.add,
                        accum_out=accS1[:, 12 + 4 * g + dc : 13 + 4 * g + dc],
                    )

                # PE: prefetch next tile's transposes ahead of the chains
                if not last:
                    xT_t = transposes(t + 1)

                # PE: ones-chains accumulate V/W/S2/Avec/Bvec
                for q, rhs in enumerate((v_t, w_t, xx, a_t, b_t)):
                    nc.tensor.matmul(
                        vws[q][:], ones[:], rhs[:],
                        start=first, stop=last,
                    )

            # evacuate PSUM d-vectors (ACT/DVE interleaved)
            for q in range(5):
                dst = evac[:, q * D : (q + 1) * D]
                if q % 2 == 0:
                    nc.scalar.activation(dst, vws[q][:], AF.Copy)
                else:
                    nc.vector.tensor_copy(dst, vws[q][:])

            nc.sync.dma_start(o_vec[:, :], evac[:])
            nc.sync.dma_start(o_s1[:, :], accS1[:])

    nc.compile()
    return nc


def get_program():
    if "nc" not in _prog_cache:
        _prog_cache["nc"] = build_program()
    return _prog_cache["nc"]


def make_in_maps(x, p_mu, p_logvar):
    x = np.ascontiguousarray(np.asarray(x, dtype=np.float32)).reshape(B, D, HW)
    p_mu = np.ascontiguousarray(np.asarray(p_mu, dtype=np.float32))
    p_logvar = np.ascontiguousarray(np.asarray(p_logvar, dtype=np.float32))
    identh = np.eye(128, dtype=np.float16)
    in_maps = []
    for c in range(NCORES):
        in_maps.append(
            {
                "x_s": np.ascontiguousarray(x[BLKB * c : BLKB * (c + 1)]),
                "mu_s": np.ascontiguousarray(p_mu[ROWS * c : ROWS * (c + 1)]),
                "lv_s": np.ascontiguousarray(p_logvar[ROWS * c : ROWS * (c + 1)]),
                "identh": identh,
            }
        )
    return in_maps


def finish_host(results):
    """Combine per-core partials (float64) into the scalar loss."""
    Vv = np.zeros(D)
    Ww = np.zeros(D)
    S2 = np.zeros(D)
    S1 = np.zeros(D)
    A = 0.0
    Bb = 0.0
    for r in results:
        vec = r["o_vec"].astype(np.float64).reshape(5, D)
        Vv += vec[0]
        Ww += vec[1]
        S2 += vec[2]
        A += float(vec[3].sum())
        Bb += float(vec[4].sum())
        s1p = r["o_s1"].astype(np.float64)  # (128, 28)
        for dc in range(NDC):
            cols = [4 * t + dc for t in range(4)] + [12 + 4 * g + dc for g in (1, 2, 3)]
            S1[128 * dc : 128 * (dc + 1)] += s1p[:, cols].sum(axis=1)
    m1 = S1 / N
    m2 = S2 / N
    S = A - 2.0 * Bb - float(np.dot(m2, Vv)) + 2.0 * float(np.dot(m1, Ww))
    return np.float32(-0.5 / N * S)


def run_on_device(x, p_mu, p_logvar, trace=False, **kw):
    from concourse import bass_utils

    nc = get_program()
    in_maps = make_in_maps(x, p_mu, p_logvar)
    return bass_utils.run_bass_kernel_spmd(
        nc, in_maps, list(range(NCORES)), trace=trace, **kw
    )


def kernel(x, p_mu, p_logvar):
    res = run_on_device(x, p_mu, p_logvar)
    return finish_host(res.results)


# revision 27
# speedup vs baseline: 1.0586x; 1.0586x over previous
"""CLUB loss kernel for Trainium2, 8-core data-parallel SPMD (i-major, fp16).

Math: with flat_x (N,D) [from x (B,D,H,W) -> (B*H*W, D)], v = exp(-p_logvar),
  loss = (-0.5/N) * [ A - 2B - dot(m2, V) + 2*dot(m1, W) ]
where
  A = sum_{i,d} x^2 v      B = sum_{i,d} x mu v
  V_d = sum_i v            W_d = sum_i mu v
  S1_d = sum_i x           S2_d = sum_i x^2     m1 = S1/N, m2 = S2/N
Per-core partials; tiny cross-core reduction + final dots on host in f64.

Layout: i-major (partition = token i, free = d).  mu/lv stream in their
natural (N, D) layout -- no transposes.  Only x (natural d-major) is
PE-transposed (64 fp16 identity matmuls/core, ~105ns each).  ALL six
reductions (V/W/S2/S1/Avec/Bvec, each a per-d column sum over i) are
ones-vector matmuls on the PE, PSUM-accumulated across the 16 i-tiles;
A/B are host sums of Avec/Bvec.

fp16 (not bf16): 8x finer mantissa at identical DVE/PE speed; all
intermediates (v<150, w<1k, a,b<6k) fit fp16 range.  HW-measured op
costs that drive this structure: DVE tensor_tensor f16 = 426ns (2x mode;
scalar_tensor_tensor is stuck at 1x/690ns, tensor_tensor_reduce crashes
the runtime), ACT activation = (N+352)/1.2 any dtype, fp16 transposes
~105ns effective, ones-MM ~220-430ns.

Engines/tile (128 i x 512 d): PE 4 transposes + 6 ones-chain MMs;
ACT exp (v, f16) + square (xx from PSUM xT, f16); DVE copy xTb (f16,
from PSUM) + w=mu*v + a=xx*v + b=w*xTb (all tensor_tensor f16 2x).

DMA: x and mu need fp32->fp16 casts so they ride the SWDGE (gpsimd)
ring, ordered by need [xh(group), mu, mu, ...]; lv (fp32, no cast) on
the two HWDGE rings (sync/scalar), throttled to consumption rate by
lv pool recycling (bufs=3) so the SWDGE ring gets the early bandwidth.
"""

import sys

import numpy as np

for _p in ("/opt/trn_rl_repo",):
    if _p not in sys.path:
        sys.path.append(_p)

B, D, H, W = 16, 512, 32, 32
HW = H * W
N = B * HW
NCORES = 8
BLKB = B // NCORES          # b-blocks per core (2)
ROWS = N // NCORES          # rows per core (2048)
NT = ROWS // 128            # 128-row i-tiles per core (16)
NDC = D // 128              # d chunks (4)
NG = 4                      # x groups per core (b-block halves), 4 tiles each

_prog_cache = {}


def build_program():
    import concourse.bacc as bacc
    import concourse.tile as tile
    from concourse import mybir

    f32 = mybir.dt.float32
    f16 = mybir.dt.float16
    AF = mybir.ActivationFunctionType
    OP = mybir.AluOpType

    nc = bacc.Bacc(
        "TRN2",
        target_bir_lowering=False,
        debug=False,
        enable_asserts=False,
        num_devices=NCORES,
    )

    x_d = nc.dram_tensor("x_s", (BLKB, D, HW), f32, kind="ExternalInput").ap()
    mu_d = nc.dram_tensor("mu_s", (ROWS, D), f32, kind="ExternalInput").ap()
    lv_d = nc.dram_tensor("lv_s", (ROWS, D), f32, kind="ExternalInput").ap()
    id_d = nc.dram_tensor("identh", (128, 128), f16, kind="ExternalInput").ap()

    # o_vec rows (after host reshape to (6, D)): [V, W, S2, S1, Avec, Bvec]
    o_vec = nc.dram_tensor("o_vec", (1, 6 * D), f32, kind="ExternalOutput").ap()

    with tile.TileContext(nc) as tc:
        with (
            tc.tile_pool(name="const", bufs=1) as constp,
            tc.tile_pool(name="xh", bufs=4) as xp,
            tc.tile_pool(name="lvsl", bufs=3) as lvp,
            tc.tile_pool(name="musl", bufs=8) as mup,
            tc.tile_pool(name="elem", bufs=3) as ep,
            tc.tile_pool(name="accum", bufs=1) as accp,
            tc.tile_pool(name="psum", bufs=2, space="PSUM") as pp,
            tc.tile_pool(name="psv", bufs=1, space="PSUM") as pvp,
        ):
            identh = constp.tile([128, 128], f16, tag="idh", name="idh")
            ones = constp.tile([128, 1], f16, tag="ones", name="ones")
            nc.vector.memset(ones[:], 1.0)

            vws = [
                pvp.tile([1, D], f32, tag=f"vws{q}", name=f"vws{q}")
                for q in range(6)
            ]
            evac = accp.tile([1, 6 * D], f32, tag="evac", name="evac")

            lv_slabs = {}   # slab s: i-tiles 2s, 2s+1 (fp32, natural)
            mu_slabs = {}   # slab s: i-tiles 2s, 2s+1 (f16 cast, natural)
            xhalf = {}      # group g: (128, NDC*512) f16, d-major half-block

            def load_lv(s, eng):
                t_ = lvp.tile([128, 2 * D], f32, tag="lv", name="lv_sl")
                rows = lv_d[256 * s : 256 * (s + 1), :]
                eng.dma_start(t_[:], rows.rearrange("(g p) f -> p g f", p=128))
                lv_slabs[s] = t_

            def load_mu(s):
                t_ = mup.tile([128, 2 * D], f16, tag="mu", name="mu_sl")
                rows = mu_d[256 * s : 256 * (s + 1), :]
                nc.gpsimd.dma_start(t_[:], rows.rearrange("(g p) f -> p g f", p=128))
                mu_slabs[s] = t_

            xtile = {}      # t (<4): (128, NDC*128) f16 single-tile x

            def load_xh(g):
                b, h = divmod(g, 2)
                t_ = xp.tile([128, NDC * 512], f16, tag="xh", name="xh")
                src = x_d[b, :, 512 * h : 512 * (h + 1)]
                nc.gpsimd.dma_start(t_[:], src.rearrange("(g p) f -> p g f", p=128))
                xhalf[g] = t_

            def load_xt(t):
                # per-tile x load (256 KB read) for the boot group: lets
                # tile t's transposes start without waiting for a full
                # 1 MB group
                t_ = xp.tile([128, NDC * 128], f16, tag="xt", name="xt")
                src = x_d[0, :, 128 * t : 128 * (t + 1)]
                nc.gpsimd.dma_start(t_[:], src.rearrange("(g p) f -> p g f", p=128))
                xtile[t] = t_

            # ---- DMA issue ----
            # SWDGE ring (casts): by-need order; HW rings: lv, throttled
            # by lvp recycling so they don't front-run the SWDGE stream.
            nc.sync.dma_start(identh[:], id_d[:])
            load_xt(0)
            load_mu(0)
            load_xt(1)
            load_xt(2)
            load_xt(3)
            load_mu(1)
            load_xh(1)
            load_mu(2)
            load_mu(3)
            load_xh(2)
            load_mu(4)
            load_mu(5)
            load_xh(3)
            load_mu(6)
            load_mu(7)
            for s in range(8):
                load_lv(s, nc.sync if s % 2 == 0 else nc.scalar)

            # ---- compute ----
            def transposes(t):
                g, jj = divmod(t, 4)
                xT = pp.tile([128, D], f16, tag="xT", name="xT")
                for dc in range(NDC):
                    if t < 4:
                        src = xtile[t][:, 128 * dc : 128 * (dc + 1)]
                    else:
                        col = 512 * dc + 128 * jj
                        src = xhalf[g][:, col : col + 128]
                    nc.tensor.matmul(
                        xT[:, 128 * dc : 128 * (dc + 1)],
                        src,
                        identh[:],
                        is_transpose=True,
                        start=(dc == 0),
                        stop=(dc == NDC - 1),
                    )
                return xT

            xT_t = transposes(0)
            for t in range(NT):
                first, last = (t == 0), (t == NT - 1)
                xT = xT_t

                # ACT: v = exp(-lv) f16; xx = (xT)^2 f16 (evacuates PSUM)
                lvsl = lv_slabs[t // 2][:, D * (t % 2) : D * (t % 2 + 1)]
                v_t = ep.tile([128, D], f16, tag="v", name="v_t")
                nc.scalar.activation(v_t[:], lvsl, AF.Exp, scale=-1.0)
                xx = ep.tile([128, D], f16, tag="xx", name="xx")
                nc.scalar.activation(xx[:], xT[:], AF.Square)

                # DVE: xTb = copy(xT); products (tensor_tensor, 2x)
                musl = mu_slabs[t // 2][:, D * (t % 2) : D * (t % 2 + 1)]
                xTb = ep.tile([128, D], f16, tag="xTb", name="xTb")
                nc.vector.tensor_copy(xTb[:], xT[:])
                w_t = ep.tile([128, D], f16, tag="w", name="w_t")
                nc.vector.tensor_tensor(w_t[:], musl, v_t[:], OP.mult)
                a_t = ep.tile([128, D], f16, tag="a", name="a_t")
                nc.vector.tensor_tensor(a_t[:], xx[:], v_t[:], OP.mult)
                b_t = ep.tile([128, D], f16, tag="b", name="b_t")
                nc.vector.tensor_tensor(b_t[:], w_t[:], xTb[:], OP.mult)

                # PE: prefetch next tile's transposes ahead of the chains
                if not last:
                    xT_t = transposes(t + 1)

                # PE: ones-chains accumulate V/W/S2/S1/Avec/Bvec
                for q, rhs in enumerate((v_t, w_t, xx, xTb, a_t, b_t)):
                    nc.tensor.matmul(
                        vws[q][:], ones[:], rhs[:],
                        start=first, stop=last,
                    )

            # evacuate PSUM d-vectors (ACT/DVE interleaved)
            for q in range(6):
                dst = evac[:, q * D : (q + 1) * D]
                if q % 2 == 0:
                    nc.scalar.activation(dst, vws[q][:], AF.Copy)
                else:
                    nc.vector.tensor_copy(dst, vws[q][:])

            nc.sync.dma_start(o_vec[:, :], evac[:])

    nc.compile()
    return nc


def get_program():
    if "nc" not in _prog_cache:
        _prog_cache["nc"] = build_program()
    return _prog_cache["nc"]


def make_in_maps(x, p_mu, p_logvar):
    x = np.ascontiguousarray(np.asarray(x, dtype=np.float32)).reshape(B, D, HW)
    p_mu = np.ascontiguousarray(np.asarray(p_mu, dtype=np.float32))
    p_logvar = np.ascontiguousarray(np.asarray(p_logvar, dtype=np.float32))
    identh = np.eye(128, dtype=np.float16)
    in_maps = []
    for c in range(NCORES):
        in_maps.append(
            {
                "x_s": np.ascontiguousarray(x[BLKB * c : BLKB * (c + 1)]),
                "mu_s": np.ascontiguousarray(p_mu[ROWS * c : ROWS * (c + 1)]),
                "lv_s": np.ascontiguousarray(p_logvar[ROWS * c : ROWS * (c + 1)]),
                "identh": identh,
            }
        )
    return in_maps


def finish_host(results):
    """Combine per-core partials (float64) into the scalar loss."""
    Vv = np.zeros(D)
    Ww = np.zeros(D)
    S2 = np.zeros(D)
    S1 = np.zeros(D)
    A = 0.0
    Bb = 0.0
    for r in results:
        vec = r["o_vec"].astype(np.float64).reshape(6, D)
        Vv += vec[0]
        Ww += vec[1]
        S2 += vec[2]
        S1 += vec[3]
        A += float(vec[4].sum())
        Bb += float(vec[5].sum())
    m1 = S1 / N
    m2 = S2 / N
    S = A - 2.0 * Bb - float(np.dot(m2, Vv)) + 2.0 * float(np.dot(m1, Ww))
    return np.float32(-0.5 / N * S)


def run_on_device(x, p_mu, p_logvar, trace=False, **kw):
    from concourse import bass_utils

    nc = get_program()
    in_maps = make_in_maps(x, p_mu, p_logvar)
    return bass_utils.run_bass_kernel_spmd(
        nc, in_maps, list(range(NCORES)), trace=trace, **kw
    )


def kernel(x, p_mu, p_logvar):
    res = run_on_device(x, p_mu, p_logvar)
    return finish_host(res.results)


# revision 32
# speedup vs baseline: 1.0894x; 1.0291x over previous
"""CLUB loss kernel for Trainium2, 8-core data-parallel SPMD (i-major, fp16).

Math: with flat_x (N,D) [from x (B,D,H,W) -> (B*H*W, D)], v = exp(-p_logvar),
  loss = (-0.5/N) * [ A - 2B - dot(m2, V) + 2*dot(m1, W) ]
where
  A = sum_{i,d} x^2 v      B = sum_{i,d} x mu v
  V_d = sum_i v            W_d = sum_i mu v
  S1_d = sum_i x           S2_d = sum_i x^2     m1 = S1/N, m2 = S2/N
Per-core partials; tiny cross-core reduction + final dots on host in f64.

Layout: i-major (partition = token i, free = d).  mu/lv stream in their
natural (N, D) layout -- no transposes.  Only x (natural d-major) is
PE-transposed (64 fp16 identity matmuls/core, ~105ns each).  ALL six
reductions (V/W/S2/S1/Avec/Bvec, each a per-d column sum over i) are
ones-vector matmuls on the PE, PSUM-accumulated across the 16 i-tiles;
A/B are host sums of Avec/Bvec.

fp16 (not bf16): 8x finer mantissa at identical DVE/PE speed; all
intermediates (v<150, w<1k, a,b<6k) fit fp16 range.  HW-measured op
costs that drive this structure: DVE tensor_tensor f16 = 426ns (2x mode;
scalar_tensor_tensor is stuck at 1x/690ns, tensor_tensor_reduce crashes
the runtime), ACT activation = (N+352)/1.2 any dtype, fp16 transposes
~105ns effective, ones-MM ~220-430ns.

Engines/tile (128 i x 512 d): PE 4 transposes + 6 ones-chain MMs;
ACT exp (v, f16) + square (xx from PSUM xT, f16); DVE copy xTb (f16,
from PSUM) + w=mu*v + a=xx*v + b=w*xTb (all tensor_tensor f16 2x).

DMA: x and mu need fp32->fp16 casts so they ride the SWDGE (gpsimd)
ring, ordered by need [xh(group), mu, mu, ...]; lv (fp32, no cast) on
the two HWDGE rings (sync/scalar), throttled to consumption rate by
lv pool recycling (bufs=3) so the SWDGE ring gets the early bandwidth.
"""

import sys

import numpy as np

for _p in ("/opt/trn_rl_repo",):
    if _p not in sys.path:
        sys.path.append(_p)

B, D, H, W = 16, 512, 32, 32
HW = H * W
N = B * HW
NCORES = 8
BLKB = B // NCORES          # b-blocks per core (2)
ROWS = N // NCORES          # rows per core (2048)
NT = ROWS // 128            # 128-row i-tiles per core (16)
NDC = D // 128              # d chunks (4)
NG = 4                      # x groups per core (b-block halves), 4 tiles each

_prog_cache = {}


def build_program():
    import concourse.bacc as bacc
    import concourse.tile as tile
    from concourse import mybir

    f32 = mybir.dt.float32
    f16 = mybir.dt.float16
    AF = mybir.ActivationFunctionType
    OP = mybir.AluOpType

    nc = bacc.Bacc(
        "TRN2",
        target_bir_lowering=False,
        debug=False,
        enable_asserts=False,
        num_devices=NCORES,
    )

    x_d = nc.dram_tensor("x_s", (BLKB, D, HW), f32, kind="ExternalInput").ap()
    mu_d = nc.dram_tensor("mu_s", (ROWS, D), f32, kind="ExternalInput").ap()
    lv_d = nc.dram_tensor("lv_s", (ROWS, D), f32, kind="ExternalInput").ap()
    id_d = nc.dram_tensor("identh", (128, 128), f16, kind="ExternalInput").ap()

    # o_vec rows (after host reshape to (6, D)): [V, W, S2, S1, Avec, Bvec]
    o_vec = nc.dram_tensor("o_vec", (1, 6 * D), f32, kind="ExternalOutput").ap()

    with tile.TileContext(nc) as tc:
        with (
            tc.tile_pool(name="const", bufs=1) as constp,
            tc.tile_pool(name="xh", bufs=4) as xp,
            tc.tile_pool(name="lvsl", bufs=4) as lvp,
            tc.tile_pool(name="musl", bufs=8) as mup,
            tc.tile_pool(name="elem", bufs=3) as ep,
            tc.tile_pool(name="accum", bufs=1) as accp,
            tc.tile_pool(name="psum", bufs=4, space="PSUM") as pp,
            tc.tile_pool(name="psv", bufs=1, space="PSUM") as pvp,
        ):
            identh = constp.tile([128, 128], f16, tag="idh", name="idh")
            ones = constp.tile([128, 1], f16, tag="ones", name="ones")
            nc.vector.memset(ones[:], 1.0)

            # chain accumulators: col-tiled so 4 chain MMs run concurrently
            # in distinct 32-col PE groups; bank A rows 0/32/64/96 hold
            # V/W/S2/S1, bank B rows 0/32 hold Avec/Bvec
            vwsA = pvp.tile([128, D], f32, tag="vwsA", name="vwsA")
            vwsB = pvp.tile([128, D], f32, tag="vwsB", name="vwsB")

            def vq(q):
                if q < 4:
                    return vwsA[32 * q : 32 * q + 1, :]
                return vwsB[32 * (q - 4) : 32 * (q - 4) + 1, :]

            evac = accp.tile([1, 6 * D], f32, tag="evac", name="evac")

            lv_slabs = {}   # slab s: i-tiles 2s, 2s+1 (fp32, natural)
            mu_slabs = {}   # slab s: i-tiles 2s, 2s+1 (f16 cast, natural)
            xhalf = {}      # group g: (128, NDC*512) f16, d-major half-block

            def load_lv(s, eng):
                t_ = lvp.tile([128, 2 * D], f32, tag="lv", name="lv_sl")
                rows = lv_d[256 * s : 256 * (s + 1), :]
                eng.dma_start(t_[:], rows.rearrange("(g p) f -> p g f", p=128))
                lv_slabs[s] = t_

            def load_mu(s):
                t_ = mup.tile([128, 2 * D], f16, tag="mu", name="mu_sl")
                rows = mu_d[256 * s : 256 * (s + 1), :]
                nc.gpsimd.dma_start(t_[:], rows.rearrange("(g p) f -> p g f", p=128))
                mu_slabs[s] = t_

            xtile = {}      # t (<4): (128, NDC*128) f16 single-tile x

            def load_xh(g):
                b, h = divmod(g, 2)
                t_ = xp.tile([128, NDC * 512], f16, tag="xh", name="xh")
                src = x_d[b, :, 512 * h : 512 * (h + 1)]
                nc.gpsimd.dma_start(t_[:], src.rearrange("(g p) f -> p g f", p=128))
                xhalf[g] = t_

            def load_xt(t):
                # per-tile x load (256 KB read) for the boot group: lets
                # tile t's transposes start without waiting for a full
                # 1 MB group
                t_ = xp.tile([128, NDC * 128], f16, tag="xt", name="xt")
                src = x_d[0, :, 128 * t : 128 * (t + 1)]
                nc.gpsimd.dma_start(t_[:], src.rearrange("(g p) f -> p g f", p=128))
                xtile[t] = t_

            # ---- DMA issue ----
            # SWDGE ring (casts): by-need order; HW rings: lv, throttled
            # by lvp recycling so they don't front-run the SWDGE stream.
            nc.sync.dma_start(identh[:], id_d[:])
            load_xt(0)
            load_mu(0)
            load_xt(1)
            load_xt(2)
            load_xt(3)
            load_mu(1)
            load_xh(1)
            load_mu(2)
            load_mu(3)
            load_xh(2)
            load_mu(4)
            load_mu(5)
            load_xh(3)
            load_mu(6)
            load_mu(7)
            for s in range(8):
                load_lv(s, nc.sync if s % 2 == 0 else nc.scalar)

            # ---- compute ----
            def transposes(t):
                g, jj = divmod(t, 4)
                xT = pp.tile([128, D], f16, tag="xT", name="xT")
                for dc in range(NDC):
                    if t < 4:
                        src = xtile[t][:, 128 * dc : 128 * (dc + 1)]
                    else:
                        col = 512 * dc + 128 * jj
                        src = xhalf[g][:, col : col + 128]
                    nc.tensor.matmul(
                        xT[:, 128 * dc : 128 * (dc + 1)],
                        src,
                        identh[:],
                        is_transpose=True,
                        start=(dc == 0),
                        stop=(dc == NDC - 1),
                    )
                return xT

            xT_t = transposes(0)
            for t in range(NT):
                first, last = (t == 0), (t == NT - 1)
                xT = xT_t

                # ACT: v = exp(-lv) f16; xx = (xT)^2 f16 (evacuates PSUM)
                lvsl = lv_slabs[t // 2][:, D * (t % 2) : D * (t % 2 + 1)]
                v_t = ep.tile([128, D], f16, tag="v", name="v_t")
                nc.scalar.activation(v_t[:], lvsl, AF.Exp, scale=-1.0)
                xx = ep.tile([128, D], f16, tag="xx", name="xx")
                nc.scalar.activation(xx[:], xT[:], AF.Square)

                # DVE: xTb = copy(xT); products (tensor_tensor, 2x)
                musl = mu_slabs[t // 2][:, D * (t % 2) : D * (t % 2 + 1)]
                xTb = ep.tile([128, D], f16, tag="xTb", name="xTb")
                nc.vector.tensor_copy(xTb[:], xT[:])
                w_t = ep.tile([128, D], f16, tag="w", name="w_t")
                nc.vector.tensor_tensor(w_t[:], musl, v_t[:], OP.mult)
                a_t = ep.tile([128, D], f16, tag="a", name="a_t")
                nc.vector.tensor_tensor(a_t[:], xx[:], v_t[:], OP.mult)
                b_t = ep.tile([128, D], f16, tag="b", name="b_t")
                nc.vector.tensor_tensor(b_t[:], w_t[:], xTb[:], OP.mult)

                # PE: prefetch next tile's transposes ahead of the chains
                if not last:
                    xT_t = transposes(t + 1)

                # PE: ones-chains accumulate V/W/S2/S1/Avec/Bvec
                # (col-tiled: 4 concurrent, then 2)
                for q, rhs in enumerate((v_t, w_t, xx, xTb, a_t, b_t)):
                    nc.tensor.matmul(
                        vq(q), ones[:], rhs[:],
                        start=first, stop=last,
                        tile_position=(0, 32 * (q % 4)),
                    )

            # evacuate PSUM d-vectors (ACT/DVE interleaved)
            for q in range(6):
                dst = evac[:, q * D : (q + 1) * D]
                if q % 2 == 0:
                    nc.scalar.activation(dst, vq(q), AF.Copy)
                else:
                    nc.vector.tensor_copy(dst, vq(q))

            nc.sync.dma_start(o_vec[:, :], evac[:])

    nc.compile()
    return nc


def get_program():
    if "nc" not in _prog_cache:
        _prog_cache["nc"] = build_program()
    return _prog_cache["nc"]


def make_in_maps(x, p_mu, p_logvar):
    x = np.ascontiguousarray(np.asarray(x, dtype=np.float32)).reshape(B, D, HW)
    p_mu = np.ascontiguousarray(np.asarray(p_mu, dtype=np.float32))
    p_logvar = np.ascontiguousarray(np.asarray(p_logvar, dtype=np.float32))
    identh = np.eye(128, dtype=np.float16)
    in_maps = []
    for c in range(NCORES):
        in_maps.append(
            {
                "x_s": np.ascontiguousarray(x[BLKB * c : BLKB * (c + 1)]),
                "mu_s": np.ascontiguousarray(p_mu[ROWS * c : ROWS * (c + 1)]),
                "lv_s": np.ascontiguousarray(p_logvar[ROWS * c : ROWS * (c + 1)]),
                "identh": identh,
            }
        )
    return in_maps


def finish_host(results):
    """Combine per-core partials (float64) into the scalar loss."""
    Vv = np.zeros(D)
    Ww = np.zeros(D)
    S2 = np.zeros(D)
    S1 = np.zeros(D)
    A = 0.0
    Bb = 0.0
    for r in results:
        vec = r["o_vec"].astype(np.float64).reshape(6, D)
        Vv += vec[0]
        Ww += vec[1]
        S2 += vec[2]
        S1 += vec[3]
        A += float(vec[4].sum())
        Bb += float(vec[5].sum())
    m1 = S1 / N
    m2 = S2 / N
    S = A - 2.0 * Bb - float(np.dot(m2, Vv)) + 2.0 * float(np.dot(m1, Ww))
    return np.float32(-0.5 / N * S)


def run_on_device(x, p_mu, p_logvar, trace=False, **kw):
    from concourse import bass_utils

    nc = get_program()
    in_maps = make_in_maps(x, p_mu, p_logvar)
    return bass_utils.run_bass_kernel_spmd(
        nc, in_maps, list(range(NCORES)), trace=trace, **kw
    )


def kernel(x, p_mu, p_logvar):
    res = run_on_device(x, p_mu, p_logvar)
    return finish_host(res.results)
